# revision 1
# baseline (speedup 1.0000x reference)
"""Gumbel-Sinkhorn straight-through kernel for Trainium2 (raw Bass, manual sems).

Math: the reference computes, per sample matrix, L = (sigmoid(gamma)+noise)/temp,
then 20 iterations of row-logsumexp-subtract / col-logsumexp-subtract, and
returns exp(result).  In linear space that is exactly Sinkhorn scaling:
    X0 = exp(L - rowmax-ish shift)      (shift cancels in the first row norm)
    repeat 20x:  X /= rowsum(X);  X /= colsum(X)
which is what runs on device (fp32 throughout; the shift keeps exp in range).

Sharding: pure data parallel over samples -> 1024 per core, SPMD on 8 cores.

Per-core layout: two resident halves X_h[128, 256, 64] fp32 in SBUF.
Partition p = (hh, i): two blocks of 64 matrix-rows; free = (seg, j).
Sample s = h*512 + hh*256 + seg.

Engines, per half-iteration:
  DVE  rowscale X *= A (A bcast over j)     - 1x fp32 tensor_tensor
  PE   colsums: block-diag-ones lhsT @ X    - output replicated over i (PSUM)
  ACT  B = exp(-ln(colsum))                 - reciprocal (ACT Reciprocal is
                                              banned in bass; exp/ln share one
                                              activation table set)
  DVE  colscale X[:, chunk] *= B
  PE   rowsums: 64 accumulating identity-weighted matmuls (one per j)
  ACT  A' = exp(-ln(rowsum))
DVE (2 full passes/iter at 1 elem/cycle/lane) is the bottleneck; PE and ACT
hide under it.  Raw Bass (not Tile) because this toolchain's walrus supports
only a single sync-wait command per compute instruction - Tile's multi-wait
tail drain cannot compile, while manual sems give true transitivity through
semaphore chains with one wait per instruction.
"""

import sys

if "/opt/trn_rl_repo" not in sys.path:
    sys.path.insert(0, "/opt/trn_rl_repo")

import numpy as np

N = 64
ITERS = 20
TEMP = 0.1
NUM_SAMPLES = 8192
NCORES = 8
S_PER_CORE = NUM_SAMPLES // NCORES  # 1024

_PROGRAM_CACHE = {}


def _bc(ap, idx, count):
    """AP with a stride-0 (broadcast) free dim of `count` inserted at free
    position idx."""
    import concourse.bass as bass

    dims = list(ap.ap)
    dims.insert(1 + idx, [0, count])
    return bass.AP(tensor=ap.tensor, offset=ap.offset, ap=dims)


def build_program(s_per_core=S_PER_CORE, sub_segs=16, iters=ITERS, nb=4):
    from contextlib import ExitStack

    import concourse.bass as bass
    from concourse import mybir

    f32 = mybir.dt.float32
    AF = mybir.ActivationFunctionType

    assert s_per_core % 4 == 0
    half = s_per_core // 2
    nseg = half // 2
    assert nseg % sub_segs == 0
    nsub = nseg // sub_segs
    assert sub_segs % 8 == 0
    mm_per_sub = sub_segs // 8  # one N=512 fp32 matmul covers 8 segs

    nc = bass.Bass()
    noise_d = nc.dram_tensor("noise", [s_per_core, N, N], f32, kind="ExternalInput")
    consts_d = nc.dram_tensor("consts", [128, N + 256], f32, kind="ExternalInput")
    out_d = nc.dram_tensor("out", [s_per_core, N, N], f32, kind="ExternalOutput")

    def dram_ap(tensor_d, h, hh):
        base = (h * half + hh * nseg) * N * N
        return bass.AP(
            tensor=tensor_d.tensor if hasattr(tensor_d, "tensor") else tensor_d,
            offset=base,
            ap=[[N, N], [N * N, nseg], [1, N]],
        )

    # ---------------- tick schedules (prefix counts per engine) ----------
    # DVE: pre h: [redmax, ttsub, ttmulE]; iter (t,h): [rowscale, colscale*nsub]
    dve_ttsub = {h: 3 * h + 2 for h in range(2)}
    dve_ttmulE = {h: 3 * h + 3 for h in range(2)}

    def dve_rowscale(t, h):
        return 6 + (2 * t + h) * (1 + nsub) + 1

    def dve_colscale(t, h, n):
        return 6 + (2 * t + h) * (1 + nsub) + 2 + n

    # ACT: [exp0, exp1, lnR0, expA0, lnR1, expA1] then per (t,h):
    #      [(lnC, expB)*nsub, (t<last: lnR, expA)]
    act_exp = {0: 1, 1: 2}
    act_expA_pre = {0: 4, 1: 6}

    # simulate ACT counter
    act_expB = {}
    act_expA = {}
    _a = 6
    for _t in range(iters):
        for _h in range(2):
            for _n in range(nsub):
                _a += 1  # lnC
                _a += 1  # expB
                act_expB[(_t, _h, _n)] = _a
            if _t < iters - 1:
                _a += 1  # lnR
                _a += 1  # expA
                act_expA[(_t + 1, _h)] = _a
    act_expA.update({(0, 0): act_expA_pre[0], (0, 1): act_expA_pre[1]})

    # PE: [rowsums_pre0, rowsums_pre1]; per (t,h): [colsum*nsub, (t<last: rowsums)]
    pe_rowsum = {("pre", 0): 1, ("pre", 1): 2}
    pe_colsum = {}
    _p = 2
    for _t in range(iters):
        for _h in range(2):
            for _n in range(nsub):
                _p += 1
                pe_colsum[(_t, _h, _n)] = _p
            if _t < iters - 1:
                _p += 1
                pe_rowsum[(_t, _h)] = _p

    with ExitStack() as ctx:
        e = ctx.enter_context
        X = [e(nc.sbuf_tensor(f"x{h}", [128, nseg, N], f32)) for h in range(2)]
        A = [e(nc.sbuf_tensor(f"a{h}", [128, nseg], f32)) for h in range(2)]
        consts = e(nc.sbuf_tensor("consts_sb", [128, N + 256], f32))
        B = [
            e(nc.sbuf_tensor(f"b{k}", [128, sub_segs, N], f32)) for k in range(nb)
        ]
        C = [e(nc.psum_tensor(f"c{k}", [128, sub_segs, N], f32)) for k in range(2)]
        R = [e(nc.psum_tensor(f"r{h}", [128, nseg], f32)) for h in range(2)]

        sem_in_c = e(nc.semaphore("sem_in_c"))
        sem_in_h = [e(nc.semaphore(f"sem_in_h{h}")) for h in range(2)]
        sem_out = e(nc.semaphore("sem_out"))
        sem_dve = e(nc.semaphore("sem_dve"))
        sem_act = e(nc.semaphore("sem_act"))
        sem_pe = e(nc.semaphore("sem_pe"))

        e10sg = consts[:, 0:N]
        ident = consts[:, N : N + 128]
        bd = consts[:, N + 128 : N + 256]

        with nc.Block() as block:

            @block.sync
            def _(sync):
                sync.dma_start(out=consts[:, :], in_=consts_d[:, :]).then_inc(
                    sem_in_c, 16
                )
                for h in range(2):
                    for hh in range(2):
                        sync.dma_start(
                            out=X[h][hh * 64 : (hh + 1) * 64, :, :],
                            in_=dram_ap(noise_d, h, hh),
                        ).then_inc(sem_in_h[h], 16)
                for h in range(2):
                    sync.wait_ge(sem_dve, dve_colscale(iters - 1, h, nsub - 1))
                    for hh in range(2):
                        sync.dma_start(
                            out=dram_ap(out_d, h, hh),
                            in_=X[h][hh * 64 : (hh + 1) * 64, :, :],
                        ).then_inc(sem_out, 16)
                sync.wait_ge(sem_out, 64)

            @block.vector
            def _(vector):
                # dve self-tick: explicit same-engine ordering (the sim's race
                # detector does not assume the DVE per-op drain serializes)
                dc = [0]

                def dve_self_wait():
                    if dc[0]:
                        vector.wait_ge(sem_dve, dc[0])

                def dve_inc(inst):
                    inst.then_inc(sem_dve, 1)
                    dc[0] += 1

                for h in range(2):
                    vector.wait_ge(sem_in_h[h], 32)
                    # M = rowmax(noise) into A[h] (scratch use)
                    dve_self_wait()
                    dve_inc(nc.vector.reduce_max(
                        out=A[h][:, :], in_=X[h][:, :, :], axis=mybir.AxisListType.X
                    ))
                    # X -= M (bcast over j)
                    dve_self_wait()
                    dve_inc(nc.vector.tensor_sub(
                        X[h][:, :, :], X[h][:, :, :], _bc(A[h][:, :], 1, N)
                    ))
                    if h == 0:
                        vector.wait_ge(sem_in_c, 16)
                    # after ACT exp: X *= exp(10*sigmoid(gamma)) (bcast over seg)
                    vector.wait_ge(sem_act, act_exp[h])
                    dve_inc(nc.vector.tensor_mul(
                        X[h][:, :, :], X[h][:, :, :], _bc(e10sg, 0, nseg)
                    ))
                for t in range(iters):
                    for h in range(2):
                        vector.wait_ge(sem_act, act_expA[(t, h)])
                        dve_self_wait()
                        dve_inc(nc.vector.tensor_mul(
                            X[h][:, :, :], X[h][:, :, :], _bc(A[h][:, :], 1, N)
                        ))
                        for n in range(nsub):
                            vector.wait_ge(sem_act, act_expB[(t, h, n)])
                            dve_self_wait()
                            s0 = n * sub_segs
                            dve_inc(nc.vector.tensor_mul(
                                X[h][:, s0 : s0 + sub_segs, :],
                                X[h][:, s0 : s0 + sub_segs, :],
                                B[n % nb][:, :, :],
                            ))

            @block.scalar
            def _(scalar):
                ac = [0]

                def act_self_wait():
                    if ac[0]:
                        scalar.wait_ge(sem_act, ac[0])

                def act_inc(inst):
                    inst.then_inc(sem_act, 1)
                    ac[0] += 1

                for h in range(2):
                    scalar.wait_ge(sem_dve, dve_ttsub[h])
                    act_self_wait()
                    act_inc(nc.scalar.activation(
                        out=X[h][:, :, :], in_=X[h][:, :, :], func=AF.Exp, scale=10.0
                    ))
                for h in range(2):
                    scalar.wait_ge(sem_pe, pe_rowsum[("pre", h)])
                    act_self_wait()
                    act_inc(nc.scalar.activation(
                        out=R[h][:, :], in_=R[h][:, :], func=AF.Ln
                    ))
                    act_self_wait()
                    act_inc(nc.scalar.activation(
                        out=A[h][:, :], in_=R[h][:, :], func=AF.Exp, scale=-1.0
                    ))
                # track last DVE colscale tick that read each B buffer
                b_last_read = [0] * nb
                for t in range(iters):
                    for h in range(2):
                        for n in range(nsub):
                            scalar.wait_ge(sem_pe, pe_colsum[(t, h, n)])
                            if b_last_read[n % nb]:
                                scalar.wait_ge(sem_dve, b_last_read[n % nb])
                            act_self_wait()
                            act_inc(nc.scalar.activation(
                                out=C[n % 2][:, :, :],
                                in_=C[n % 2][:, :, :],
                                func=AF.Ln,
                            ))
                            act_self_wait()
                            act_inc(nc.scalar.activation(
                                out=B[n % nb][:, :, :],
                                in_=C[n % 2][:, :, :],
                                func=AF.Exp,
                                scale=-1.0,
                            ))
                            b_last_read[n % nb] = dve_colscale(t, h, n)
                        if t < iters - 1:
                            scalar.wait_ge(sem_pe, pe_rowsum[(t, h)])
                            act_self_wait()
                            act_inc(nc.scalar.activation(
                                out=R[h][:, :], in_=R[h][:, :], func=AF.Ln
                            ))
                            act_self_wait()
                            act_inc(nc.scalar.activation(
                                out=A[h][:, :], in_=R[h][:, :], func=AF.Exp, scale=-1.0
                            ))

            @block.tensor
            def _(tensor):
                tensor.wait_ge(sem_in_c, 16)

                def rowsums(h):
                    for j in range(N):
                        mm = nc.tensor.matmul(
                            R[h][:, :],
                            ident,
                            X[h][:, :, j],
                            start=(j == 0),
                            stop=(j == N - 1),
                        )
                    mm.then_inc(sem_pe, 1)

                for h in range(2):
                    tensor.wait_ge(sem_dve, dve_ttmulE[h])
                    rowsums(h)
                # last ACT expB tick that read each C buffer
                c_last_read = [0, 0]
                for t in range(iters):
                    for h in range(2):
                        tensor.wait_ge(sem_dve, dve_rowscale(t, h))
                        for n in range(nsub):
                            if c_last_read[n % 2]:
                                tensor.wait_ge(sem_act, c_last_read[n % 2])
                            s0 = n * sub_segs
                            for m in range(mm_per_sub):
                                mm = nc.tensor.matmul(
                                    C[n % 2][:, m * 8 : (m + 1) * 8, :],
                                    bd,
                                    X[h][:, s0 + m * 8 : s0 + (m + 1) * 8, :],
                                    start=True,
                                    stop=True,
                                )
                            mm.then_inc(sem_pe, 1)
                            c_last_read[n % 2] = act_expB[(t, h, n)]
                        if t < iters - 1:
                            tensor.wait_ge(sem_dve, dve_colscale(t, h, nsub - 1))
                            rowsums(h)

    return nc


def host_constants(gamma):
    """[128, 64+256] packed: exp(10*sigmoid(gamma)) | identity | block-diag."""
    sg = 1.0 / (1.0 + np.exp(-gamma.astype(np.float64)))
    e64 = np.exp(sg / TEMP).astype(np.float32)
    e10sg = np.concatenate([e64, e64], axis=0)
    ident = np.eye(128, dtype=np.float32)
    bdiag = np.kron(np.eye(2, dtype=np.float32), np.ones((64, 64), np.float32))
    return np.concatenate([e10sg, ident, bdiag], axis=1)


def kernel(gamma: np.ndarray, gumbel_noise: np.ndarray) -> np.ndarray:
    from concourse.bass_utils import run_bass_kernel_spmd

    gamma = np.asarray(gamma, dtype=np.float32)
    noise = np.asarray(gumbel_noise, dtype=np.float32)
    s = noise.shape[0]
    s_per_core = s // NCORES
    if s_per_core not in _PROGRAM_CACHE:
        _PROGRAM_CACHE[s_per_core] = build_program(s_per_core=s_per_core)
    nc = _PROGRAM_CACHE[s_per_core]

    consts = host_constants(gamma)
    in_maps = []
    for c in range(NCORES):
        shard = np.ascontiguousarray(noise[c * s_per_core : (c + 1) * s_per_core])
        in_maps.append({"noise": shard, "consts": consts})
    res = run_bass_kernel_spmd(nc, in_maps, list(range(NCORES)))
    out = np.concatenate([r["out"] for r in res.results], axis=0)
    return out.astype(np.float32)



# revision 17
# speedup vs baseline: 8507.3400x; 8507.3400x over previous
"""Gumbel-Sinkhorn kernel for Trainium2 (raw Bass, manual sems) — v3.

Math (per sample): L = (sigmoid(gamma)+noise)/temp; 20x row/col normalize in
log space; exp at the end. In linear space with a GLOBAL shift S=80 (any
constant shift cancels in the first row normalization):
    X0 = exp(10*noise - 80) * G,  G = exp(10*sigmoid(gamma))
    repeat 20x:  X /= rowsum(X);  X /= colsum(X)

Layout (sample-per-partition): each core gets 1024 samples as 8 blocks of
128; partition p = sample-in-block, free = (i, j), j innermost (natural DMA
order). X is bf16 after the first row normalization; sums accumulate fp32 in
PSUM; reciprocals are compact [p, 64] per block.

Dynamic range: iteration-0 entries span up to e^68 (fp32-only, and beyond
the Scalar-engine Ln range of 2^64), so the pre-phase runs per-block in
fp32: exp (ACT, in-place in the staging buffer) -> *G (DVE) -> fp32 rowsums
(PE) -> exact DVE reciprocal -> rowscale0 with bf16 output. After that, rows
sum to 1, all later sums lie in (0, 64], and ACT Ln/Exp recips are safe.

Per iteration, per pair of blocks (4 pairs):
  PE   rowsums: 64 identity-matmuls accumulating X[:, :, j] -> RS [p, 128]
  ACT  ln(RS) -> LR;  exp(-LR) expanded x8 -> A8 bf16  (compact recips)
  DVE  rowscale: X *= A8   (bf16 packed both operands -> 2x mode)
  PE   colsums: 64 identity-matmuls accumulating X[:, i, :] -> CS
  ACT  ln(CS) -> LC;  exp(-LC) -> B bf16 (broadcast over i, innermost packed)
  DVE  colscale: X *= B    (2x mode)
Final iteration uses fp32 scale factors and writes fp32 straight to the
output staging buffer (bf16 X + bf16 scales = 1.4e-2 rel err; fp32 scales on
the last iteration -> 1.2e-2 vs the 2e-2 gate).

Raw Bass with counting semaphores; every compute op increments its engine's
sem by 1; consumers wait on the producer's global tick. DMA completions can
reorder across transfers, so input/output staging uses per-parity semaphores
(at most one outstanding transfer per parity) and a dedicated sem for the
constants.
"""

import sys

if "/opt/trn_rl_repo" not in sys.path:
    sys.path.insert(0, "/opt/trn_rl_repo")

import numpy as np

N = 64
ITERS = 20
TEMP = 0.1
NUM_SAMPLES = 8192
NCORES = 8
S_PER_CORE = NUM_SAMPLES // NCORES  # 1024
SHIFT = 80.0

_PROGRAM_CACHE = {}


def _fap(t, off, dims):
    """AP on tensor t with partition dim copied from t[:, :] and custom free
    dims (list of [stride, count] in elements)."""
    import concourse.bass as bass

    base = t if isinstance(t, bass.AP) else t[:, :]
    return bass.AP(tensor=base.tensor, offset=base.offset + off, ap=[base.ap[0]] + dims)


def build_program(s_per_core=S_PER_CORE, iters=ITERS):
    from contextlib import ExitStack

    import concourse.bass as bass
    from concourse import mybir

    f32 = mybir.dt.float32
    bf16 = mybir.dt.bfloat16
    AF = mybir.ActivationFunctionType

    nb = s_per_core // 128  # 8 blocks
    npair = nb // 2  # 4 pairs
    BLK = N * N  # 4096

    nc = bass.Bass()
    # register -SHIFT as a const AP so activation(bias=-SHIFT) resolves
    _shift_t = nc.alloc_sbuf_tensor("const-shift", [128, 1], f32)
    nc.gpsimd.memset(_shift_t.ap(), -SHIFT)
    nc.const_aps.aps[(f32, -SHIFT)] = _shift_t.ap()
    nc.all_engine_barrier()

    noise_d = nc.dram_tensor("noise", [s_per_core, N, N], f32, kind="ExternalInput")
    gconst_d = nc.dram_tensor("gconst", [128, BLK], bf16, kind="ExternalInput")
    ident_d = nc.dram_tensor("ident", [128, 128], bf16, kind="ExternalInput")
    identf_d = nc.dram_tensor("identf", [128, 128], f32, kind="ExternalInput")
    out_d = nc.dram_tensor("out", [s_per_core, N, N], f32, kind="ExternalOutput")

    def dram_ap(td, b):
        return bass.AP(
            tensor=td.tensor if hasattr(td, "tensor") else td,
            offset=b * 128 * BLK,
            ap=[[BLK, 128], [1, BLK]],
        )

    with ExitStack() as ctx:
        e = ctx.enter_context
        X = e(nc.sbuf_tensor("x", [128, nb * BLK], bf16))
        stage = [e(nc.sbuf_tensor(f"stage{k}", [128, BLK], f32)) for k in range(2)]
        G = e(nc.sbuf_tensor("g", [128, BLK], bf16))
        identsb = e(nc.sbuf_tensor("identsb", [128, 128], bf16))
        identf = e(nc.sbuf_tensor("identf_sb", [128, 128], f32))
        A8 = e(nc.sbuf_tensor("a8", [128, nb * N * 8], bf16))  # [blk, i, 8]
        Bv = e(nc.sbuf_tensor("bv", [128, nb * N], bf16))  # [blk, j]
        A8f = e(nc.sbuf_tensor("a8f", [128, nb * N * 8], f32))
        Bvf = e(nc.sbuf_tensor("bvf", [128, nb * N], f32))
        LR = e(nc.sbuf_tensor("lr", [128, nb * N], f32))  # also recip0 target
        LC = e(nc.sbuf_tensor("lc", [128, nb * N], f32))
        RS = [e(nc.psum_tensor(f"rs{pr}", [128, 128], f32)) for pr in range(npair)]
        CS = [e(nc.psum_tensor(f"cs{pr}", [128, 128], f32)) for pr in range(npair)]

        sem_in_c = e(nc.semaphore("sem_in_c"))
        sem_in_p = [e(nc.semaphore(f"sem_in_p{k}")) for k in range(2)]
        sem_out_p = [e(nc.semaphore(f"sem_out_p{k}")) for k in range(2)]
        sem_dve = e(nc.semaphore("sem_dve"))
        sem_act = e(nc.semaphore("sem_act"))
        sem_pe = e(nc.semaphore("sem_pe"))

        # ---------------- tick schedules (global per-engine counters) ------
        # ACT: exp_in(b)*8; t=0: [ln_c,exp_b]*4; t>=1: [ln_r,exp_a8]*4 then
        #      [ln_c,exp_b]*4
        act_exp_in = {b: b + 1 for b in range(nb)}
        _a = nb
        act_exp_a8, act_exp_b = {}, {}
        for _t in range(iters):
            if _t >= 1:
                for _pr in range(npair):
                    _a += 2
                    act_exp_a8[(_t, _pr)] = _a
            for _pr in range(npair):
                _a += 2
                act_exp_b[(_t, _pr)] = _a

        # DVE: per b: [gmul0, recip0, rowscale0]; t=0: colscale*4;
        #      t=1..18: rowscale*4, colscale*4; t=19: rowscale*4, cs19*8
        dve_rowscale0 = {}
        _d = 0
        for _b in range(nb):
            _d += 3
            dve_rowscale0[_b] = _d
        dve_rowscale, dve_colscale, dve_cs19 = {}, {}, {}
        for _t in range(iters):
            if _t >= 1:
                for _pr in range(npair):
                    _d += 1
                    dve_rowscale[(_t, _pr)] = _d
            if _t < iters - 1:
                for _pr in range(npair):
                    _d += 1
                    dve_colscale[(_t, _pr)] = _d
            else:
                for _b in range(nb):
                    _d += 1
                    dve_cs19[_b] = _d

        # PE: rowsum0(b)*8; t=0: colsum*4; t>=1: rowsum*4 then colsum*4
        pe_rowsum0 = {}
        _p = 0
        for _b in range(nb):
            _p += 1
            pe_rowsum0[_b] = _p
        pe_rowsum, pe_colsum = {}, {}
        for _t in range(iters):
            if _t >= 1:
                for _pr in range(npair):
                    _p += 1
                    pe_rowsum[(_t, _pr)] = _p
            for _pr in range(npair):
                _p += 1
                pe_colsum[(_t, _pr)] = _p

        with nc.Block() as block:

            @block.sync
            def _(sync):
                sync.dma_start(out=G[:, :], in_=gconst_d[:, :]).then_inc(sem_in_c, 16)
                sync.dma_start(out=identsb[:, :], in_=ident_d[:, :]).then_inc(
                    sem_in_c, 16
                )
                sync.dma_start(out=identf[:, :], in_=identf_d[:, :]).then_inc(
                    sem_in_c, 16
                )
                for b in range(nb):
                    if b >= 2:
                        sync.wait_ge(sem_dve, dve_rowscale0[b - 2])
                        sync.wait_ge(sem_in_p[b % 2], 16 * (b // 2))
                    sync.dma_start(
                        out=stage[b % 2][:, :], in_=dram_ap(noise_d, b)
                    ).then_inc(sem_in_p[b % 2], 16)
                for b in range(nb):
                    sync.wait_ge(sem_dve, dve_cs19[b])
                    if b >= 2:
                        sync.wait_ge(sem_out_p[b % 2], 16 * (b // 2))
                    sync.dma_start(
                        out=dram_ap(out_d, b), in_=stage[b % 2][:, :]
                    ).then_inc(sem_out_p[b % 2], 16)
                sync.wait_ge(sem_out_p[0], 16 * (nb // 2))
                sync.wait_ge(sem_out_p[1], 16 * (nb // 2))

            @block.scalar
            def _(scalar):
                ac = [0]

                def self_wait():
                    if ac[0]:
                        scalar.wait_ge(sem_act, ac[0])

                def inc(inst):
                    inst.then_inc(sem_act, 1)
                    ac[0] += 1

                for b in range(nb):
                    scalar.wait_ge(sem_in_p[b % 2], 16 * (b // 2 + 1))
                    self_wait()
                    inc(nc.scalar.activation(
                        out=stage[b % 2][:, :],
                        in_=stage[b % 2][:, :],
                        func=AF.Exp,
                        scale=10.0,
                        bias=-SHIFT,
                    ))
                for t in range(iters):
                    last = t == iters - 1
                    a8t, bvt = (A8f, Bvf) if last else (A8, Bv)
                    if t >= 1:
                        for pr in range(npair):
                            scalar.wait_ge(sem_pe, pe_rowsum[(t, pr)])
                            self_wait()
                            inc(nc.scalar.activation(
                                out=_fap(LR, pr * 128, [[1, 128]]),
                                in_=RS[pr][:, :],
                                func=AF.Ln,
                            ))
                            self_wait()
                            inc(nc.scalar.activation(
                                out=_fap(a8t, pr * 2 * N * 8,
                                         [[N * 8, 2], [8, N], [1, 8]]),
                                in_=_fap(LR, pr * 128, [[N, 2], [1, N], [0, 8]]),
                                func=AF.Exp,
                                scale=-1.0,
                            ))
                    for pr in range(npair):
                        scalar.wait_ge(sem_pe, pe_colsum[(t, pr)])
                        self_wait()
                        inc(nc.scalar.activation(
                            out=_fap(LC, pr * 128, [[1, 128]]),
                            in_=CS[pr][:, :],
                            func=AF.Ln,
                        ))
                        self_wait()
                        inc(nc.scalar.activation(
                            out=_fap(bvt, pr * 128, [[1, 128]]),
                            in_=_fap(LC, pr * 128, [[1, 128]]),
                            func=AF.Exp,
                            scale=-1.0,
                        ))

            @block.vector
            def _(vector):
                dc = [0]

                def self_wait():
                    if dc[0]:
                        vector.wait_ge(sem_dve, dc[0])

                def inc(inst):
                    inst.then_inc(sem_dve, 1)
                    dc[0] += 1

                def x_pair(pr, dims):
                    return _fap(X, pr * 2 * BLK, dims)

                vector.wait_ge(sem_in_c, 48)  # G loaded
                for b in range(nb):
                    pr, half = b // 2, b % 2
                    vector.wait_ge(sem_act, act_exp_in[b])
                    self_wait()
                    # gmul0: stage *= G (fp32 * bf16 -> fp32)
                    inc(nc.vector.tensor_mul(
                        _fap(stage[b % 2], 0, [[N, N], [1, N]]),
                        _fap(stage[b % 2], 0, [[N, N], [1, N]]),
                        _fap(G, 0, [[N, N], [1, N]]),
                    ))
                    # recip0: LR[b] = 1 / RS0[b]  (exact DVE reciprocal)
                    vector.wait_ge(sem_pe, pe_rowsum0[b])
                    self_wait()
                    inc(nc.vector.reciprocal(
                        _fap(LR, b * N, [[1, N]]),
                        _fap(RS[pr], half * N, [[1, N]]),
                    ))
                    # rowscale0: X[b] = stage * LR[b]  (fp32 -> bf16)
                    self_wait()
                    inc(nc.vector.tensor_mul(
                        _fap(X, b * BLK, [[N, N], [1, N]]),
                        _fap(stage[b % 2], 0, [[N, N], [1, N]]),
                        _fap(LR, b * N, [[1, N], [0, N]]),
                    ))
                for t in range(iters):
                    last = t == iters - 1
                    a8t, bvt = (A8f, Bvf) if last else (A8, Bv)
                    if t >= 1:
                        for pr in range(npair):
                            vector.wait_ge(sem_act, act_exp_a8[(t, pr)])
                            self_wait()
                            inc(nc.vector.tensor_mul(
                                x_pair(pr, [[BLK, 2], [N, N], [8, 8], [1, 8]]),
                                x_pair(pr, [[BLK, 2], [N, N], [8, 8], [1, 8]]),
                                _fap(a8t, pr * 2 * N * 8,
                                     [[N * 8, 2], [8, N], [0, 8], [1, 8]]),
                            ))
                    if not last:
                        for pr in range(npair):
                            vector.wait_ge(sem_act, act_exp_b[(t, pr)])
                            self_wait()
                            inc(nc.vector.tensor_mul(
                                x_pair(pr, [[BLK, 2], [N, N], [1, N]]),
                                x_pair(pr, [[BLK, 2], [N, N], [1, N]]),
                                _fap(bvt, pr * 128, [[N, 2], [0, N], [1, N]]),
                            ))
                    else:
                        for b in range(nb):
                            vector.wait_ge(sem_act, act_exp_b[(t, b // 2)])
                            if b >= 2:
                                vector.wait_ge(sem_out_p[b % 2], 16 * (b // 2))
                            self_wait()
                            inc(nc.vector.tensor_mul(
                                _fap(stage[b % 2], 0, [[N, N], [1, N]]),
                                _fap(X, b * BLK, [[N, N], [1, N]]),
                                _fap(bvt, b * N, [[0, N], [1, N]]),
                            ))

            @block.tensor
            def _(tensor):
                tensor.wait_ge(sem_in_c, 48)  # ident + identf + gconst loaded

                def chain(psum, half, src, src_off, rowsum, nblk, lhs=None):
                    # accumulate nblk-wide identity matmuls into
                    # psum[:, half*64*nblk : (half+1)*64*nblk]
                    out = _fap(psum, half * N * nblk, [[N, nblk], [1, N]])
                    for k in range(N):
                        if rowsum:
                            rhs = _fap(src, src_off + k, [[BLK, nblk], [N, N]])
                        else:
                            rhs = _fap(src, src_off + k * N, [[BLK, nblk], [1, N]])
                        mm = nc.tensor.matmul(
                            out, lhs if lhs is not None else identsb[:, :], rhs,
                            start=(k == 0), stop=(k == N - 1),
                        )
                    mm.then_inc(sem_pe, 1)

                for b in range(nb):
                    # rowsum0(b): fp32 chain over stage[b%2]
                    tensor.wait_ge(sem_dve, dve_rowscale0[b] - 2)  # gmul0(b)
                    chain(RS[b // 2], b % 2, stage[b % 2], 0, True, 1, lhs=identf[:, :])
                for t in range(iters):
                    if t >= 1:
                        for pr in range(npair):
                            tensor.wait_ge(sem_dve, dve_colscale[(t - 1, pr)])
                            chain(RS[pr], 0, X, pr * 2 * BLK, True, 2)
                    for pr in range(npair):
                        if t == 0:
                            tensor.wait_ge(sem_dve, dve_rowscale0[2 * pr + 1])
                        else:
                            tensor.wait_ge(sem_dve, dve_rowscale[(t, pr)])
                        chain(CS[pr], 0, X, pr * 2 * BLK, False, 2)

    return nc


def host_constants(gamma):
    import ml_dtypes

    sg = 1.0 / (1.0 + np.exp(-gamma.astype(np.float64)))
    g = np.exp(sg / TEMP).astype(np.float32).reshape(-1)  # [4096]
    gconst = np.tile(g[None, :], (128, 1)).astype(ml_dtypes.bfloat16)
    ident = np.eye(128, dtype=ml_dtypes.bfloat16)
    identf = np.eye(128, dtype=np.float32)
    return gconst, ident, identf


def make_in_maps(gamma, noise):
    s_per_core = noise.shape[0] // NCORES
    gconst, ident, identf = host_constants(gamma)
    in_maps = []
    for c in range(NCORES):
        shard = np.ascontiguousarray(noise[c * s_per_core : (c + 1) * s_per_core])
        in_maps.append(
            {"noise": shard, "gconst": gconst, "ident": ident, "identf": identf}
        )
    return in_maps


def assemble_output(results):
    out = np.concatenate([r["out"] for r in results], axis=0)
    return out.astype(np.float32)


def kernel(gamma: np.ndarray, gumbel_noise: np.ndarray) -> np.ndarray:
    from concourse.bass_utils import run_bass_kernel_spmd

    gamma = np.asarray(gamma, dtype=np.float32)
    noise = np.asarray(gumbel_noise, dtype=np.float32)
    s_per_core = noise.shape[0] // NCORES
    if s_per_core not in _PROGRAM_CACHE:
        _PROGRAM_CACHE[s_per_core] = build_program(s_per_core=s_per_core)
    nc = _PROGRAM_CACHE[s_per_core]

    res = run_bass_kernel_spmd(nc, make_in_maps(gamma, noise), list(range(NCORES)))
    return assemble_output(res.results)


# revision 20
# speedup vs baseline: 10928.0629x; 1.2845x over previous
"""Gumbel-Sinkhorn kernel for Trainium2 (raw Bass, manual sems) — v4.

Math (per sample): L = (sigmoid(gamma)+noise)/temp; 20x row/col normalize in
log space; exp at the end. In linear space with a GLOBAL shift S=80 (any
constant shift cancels in the first row normalization):
    X0 = exp(10*noise - 80) * G,  G = exp(10*sigmoid(gamma))
    repeat 20x:  X /= rowsum(X);  X /= colsum(X)

Layout (sample-per-partition): each core gets 1024 samples as 8 blocks of
128; partition p = sample-in-block, free = (i, j), j innermost (natural DMA
order). X is bf16 throughout (full-row bf16 underflow of X0 would need the
row-max Gumbel below -1.2: P ~ e^-212; entries far under their row max flush
to 0 harmlessly). Sums accumulate fp32 in PSUM; reciprocals are compact
[p, 64-per-block].

Iteration-0 rowsums reach ~2e31, beyond the Scalar-engine Ln range (2^64),
so t=0 uses the exact DVE `reciprocal` (fp32) + an ACT Copy that both
expands x8 and converts to bf16. After the first row normalization all sums
lie in (0, 64] and the ACT Ln/Exp reciprocal path is safe.

Per iteration, per pair of blocks (4 pairs):
  PE   rowsums: 64 identity-matmuls accumulating X[:, :, j] -> RS [p, 128]
  ACT  ln(RS) -> LR;  exp(-LR) expanded x8 -> A8 bf16  (compact recips)
  DVE  rowscale: X *= A8   (bf16 packed both operands -> 2x mode)
  PE   colsums: 64 identity-matmuls accumulating X[:, i, :] -> CS
  ACT  ln(CS) -> LC;  exp(-LC) -> B bf16 (broadcast over i, innermost packed)
  DVE  colscale: X *= B    (2x mode)
The Pool engine (gpsimd) takes block 7's rowscale+colscale each iteration
(DVE does blocks 0-6), balancing DVE ~30us/iter against PE ~28us/iter.
Final iteration uses fp32 scale factors and writes fp32 straight to the
output staging buffer (bf16 scales everywhere = 1.4e-2 rel err; fp32 on the
last iteration ~1.1e-2 vs the 2e-2 gate).

Raw Bass with counting semaphores; every compute op increments its engine's
sem by 1; consumers wait the producer's global tick. DMA completions can
reorder across transfers, so staging uses per-parity semaphores (at most one
outstanding transfer per parity) and a dedicated sem for constants.
"""

import sys

if "/opt/trn_rl_repo" not in sys.path:
    sys.path.insert(0, "/opt/trn_rl_repo")

import numpy as np

N = 64
ITERS = 20
TEMP = 0.1
NUM_SAMPLES = 8192
NCORES = 8
S_PER_CORE = NUM_SAMPLES // NCORES  # 1024
SHIFT = 80.0

_PROGRAM_CACHE = {}


def _fap(t, off, dims):
    """AP on tensor t with partition dim copied from t[:, :] and custom free
    dims (list of [stride, count] in elements)."""
    import concourse.bass as bass

    base = t if isinstance(t, bass.AP) else t[:, :]
    return bass.AP(tensor=base.tensor, offset=base.offset + off, ap=[base.ap[0]] + dims)


def build_program(s_per_core=S_PER_CORE, iters=ITERS):
    from contextlib import ExitStack

    import concourse.bass as bass
    from concourse import mybir

    f32 = mybir.dt.float32
    bf16 = mybir.dt.bfloat16
    AF = mybir.ActivationFunctionType

    nb = s_per_core // 128  # 8 blocks
    npair = nb // 2  # 4 pairs
    BLK = N * N  # 4096
    POOL_BLK = nb - 1  # block handled by the Pool engine (7)

    nc = bass.Bass()
    # register -SHIFT as a const AP so activation(bias=-SHIFT) resolves
    _shift_t = nc.alloc_sbuf_tensor("const-shift", [128, 1], f32)
    nc.gpsimd.memset(_shift_t.ap(), -SHIFT)
    nc.const_aps.aps[(f32, -SHIFT)] = _shift_t.ap()
    nc.all_engine_barrier()

    noise_d = nc.dram_tensor("noise", [s_per_core, N, N], f32, kind="ExternalInput")
    gconst_d = nc.dram_tensor("gconst", [128, BLK], bf16, kind="ExternalInput")
    ident_d = nc.dram_tensor("ident", [128, 128], bf16, kind="ExternalInput")
    out_d = nc.dram_tensor("out", [s_per_core, N, N], f32, kind="ExternalOutput")

    def dram_ap(td, b):
        return bass.AP(
            tensor=td.tensor if hasattr(td, "tensor") else td,
            offset=b * 128 * BLK,
            ap=[[BLK, 128], [1, BLK]],
        )

    with ExitStack() as ctx:
        e = ctx.enter_context
        X = e(nc.sbuf_tensor("x", [128, nb * BLK], bf16))
        stage = [e(nc.sbuf_tensor(f"stage{k}", [128, BLK], f32)) for k in range(2)]
        G = e(nc.sbuf_tensor("g", [128, BLK], bf16))
        identsb = e(nc.sbuf_tensor("identsb", [128, 128], bf16))
        A8 = e(nc.sbuf_tensor("a8", [128, nb * N * 8], bf16))  # [blk, i, 8]
        Bv = e(nc.sbuf_tensor("bv", [128, nb * N], bf16))  # [blk, j]
        A8f = e(nc.sbuf_tensor("a8f", [128, nb * N * 8], f32))
        Bvf = e(nc.sbuf_tensor("bvf", [128, nb * N], f32))
        LR = e(nc.sbuf_tensor("lr", [128, nb * N], f32))  # ln(RS) / recip0
        LC = e(nc.sbuf_tensor("lc", [128, nb * N], f32))
        RS = [e(nc.psum_tensor(f"rs{pr}", [128, 128], f32)) for pr in range(npair)]
        CS = [e(nc.psum_tensor(f"cs{pr}", [128, 128], f32)) for pr in range(npair)]

        sem_in_c = e(nc.semaphore("sem_in_c"))
        sem_in_p = [e(nc.semaphore(f"sem_in_p{k}")) for k in range(2)]
        sem_out_p = [e(nc.semaphore(f"sem_out_p{k}")) for k in range(2)]
        sem_dve = e(nc.semaphore("sem_dve"))
        sem_act = e(nc.semaphore("sem_act"))
        sem_pe = e(nc.semaphore("sem_pe"))
        sem_pool = e(nc.semaphore("sem_pool"))

        # ---------------- tick schedules (global per-engine counters) ------
        # ACT: exp_in(b)*8; t=0: [recip-copy_a8]*4 then [ln_c,exp_b]*4;
        #      t>=1: [ln_r,exp_a8]*4 then [ln_c,exp_b]*4
        act_exp_in = {b: b + 1 for b in range(nb)}
        _a = nb
        act_exp_a8, act_exp_b = {}, {}
        for _t in range(iters):
            for _pr in range(npair):
                _a += 1 if _t == 0 else 2  # t=0: Copy only; else ln_r+exp_a8
                act_exp_a8[(_t, _pr)] = _a
            for _pr in range(npair):
                _a += 2
                act_exp_b[(_t, _pr)] = _a

        # DVE: gmul(pr)*4; t=0: recip0*4 (before rowscales);
        #      each t: rowscale pairs 0-2 + single blk6, colscale same
        #      (t=19: colscale -> cs19(b)*8, all blocks on DVE)
        # DVE units per t: [recip0*4 if t==0] [rowscale(0..2,6)] [colscale...]
        dve_gmul = {pr: pr + 1 for pr in range(npair)}
        _d = npair
        dve_recip0 = {}
        dve_rowscale = {}  # (t, unit) unit in 0..3: pairs 0,1,2 then block 6
        dve_colscale = {}
        dve_cs19 = {}
        for _t in range(iters):
            if _t == 0:
                for _pr in range(npair):
                    _d += 1
                    dve_recip0[_pr] = _d
            for _u in range(npair):
                _d += 1
                dve_rowscale[(_t, _u)] = _d
            if _t < iters - 1:
                for _u in range(npair):
                    _d += 1
                    dve_colscale[(_t, _u)] = _d
            else:
                for _b in range(nb):
                    _d += 1
                    dve_cs19[_b] = _d

        # Pool: per t in 0..18: rowscale7, colscale7; t=19: rowscale7 only
        pool_rowscale7, pool_colscale7 = {}, {}
        _q = 0
        for _t in range(iters):
            _q += 1
            pool_rowscale7[_t] = _q
            if _t < iters - 1:
                _q += 1
                pool_colscale7[_t] = _q

        # PE: per t: rowsum(pr)*4 then colsum(pr)*4
        pe_rowsum, pe_colsum = {}, {}
        _p = 0
        for _t in range(iters):
            for _pr in range(npair):
                _p += 1
                pe_rowsum[(_t, _pr)] = _p
            for _pr in range(npair):
                _p += 1
                pe_colsum[(_t, _pr)] = _p

        # helper: DVE tick that completes the scale op covering block b
        def dve_row_tick(t, b):
            return dve_rowscale[(t, min(b // 2, npair - 1))]

        def dve_col_tick(t, b):
            return dve_colscale[(t, min(b // 2, npair - 1))]

        with nc.Block() as block:

            @block.sync
            def _(sync):
                sync.dma_start(out=G[:, :], in_=gconst_d[:, :]).then_inc(sem_in_c, 16)
                sync.dma_start(out=identsb[:, :], in_=ident_d[:, :]).then_inc(
                    sem_in_c, 16
                )
                for b in range(nb):
                    if b >= 2:
                        sync.wait_ge(sem_act, act_exp_in[b - 2])
                        sync.wait_ge(sem_in_p[b % 2], 16 * (b // 2))
                    sync.dma_start(
                        out=stage[b % 2][:, :], in_=dram_ap(noise_d, b)
                    ).then_inc(sem_in_p[b % 2], 16)
                for b in range(nb):
                    sync.wait_ge(sem_dve, dve_cs19[b])
                    if b >= 2:
                        sync.wait_ge(sem_out_p[b % 2], 16 * (b // 2))
                    sync.dma_start(
                        out=dram_ap(out_d, b), in_=stage[b % 2][:, :]
                    ).then_inc(sem_out_p[b % 2], 16)
                sync.wait_ge(sem_out_p[0], 16 * (nb // 2))
                sync.wait_ge(sem_out_p[1], 16 * (nb // 2))

            @block.scalar
            def _(scalar):
                ac = [0]

                def self_wait():
                    if ac[0]:
                        scalar.wait_ge(sem_act, ac[0])

                def inc(inst):
                    inst.then_inc(sem_act, 1)
                    ac[0] += 1

                for b in range(nb):
                    scalar.wait_ge(sem_in_p[b % 2], 16 * (b // 2 + 1))
                    self_wait()
                    inc(nc.scalar.activation(
                        out=_fap(X, b * BLK, [[1, BLK]]),
                        in_=stage[b % 2][:, :],
                        func=AF.Exp,
                        scale=10.0,
                        bias=-SHIFT,
                    ))
                for t in range(iters):
                    last = t == iters - 1
                    a8t, bvt = (A8f, Bvf) if last else (A8, Bv)
                    for pr in range(npair):
                        if t == 0:
                            # recip0 (DVE) wrote LR; expand x8 + bf16-convert
                            scalar.wait_ge(sem_dve, dve_recip0[pr])
                            self_wait()
                            inc(nc.scalar.activation(
                                out=_fap(A8, pr * 2 * N * 8,
                                         [[N * 8, 2], [8, N], [1, 8]]),
                                in_=_fap(LR, pr * 128, [[N, 2], [1, N], [0, 8]]),
                                func=AF.Copy,
                            ))
                        else:
                            scalar.wait_ge(sem_pe, pe_rowsum[(t, pr)])
                            self_wait()
                            inc(nc.scalar.activation(
                                out=_fap(LR, pr * 128, [[1, 128]]),
                                in_=RS[pr][:, :],
                                func=AF.Ln,
                            ))
                            self_wait()
                            inc(nc.scalar.activation(
                                out=_fap(a8t, pr * 2 * N * 8,
                                         [[N * 8, 2], [8, N], [1, 8]]),
                                in_=_fap(LR, pr * 128, [[N, 2], [1, N], [0, 8]]),
                                func=AF.Exp,
                                scale=-1.0,
                            ))
                    for pr in range(npair):
                        scalar.wait_ge(sem_pe, pe_colsum[(t, pr)])
                        self_wait()
                        inc(nc.scalar.activation(
                            out=_fap(LC, pr * 128, [[1, 128]]),
                            in_=CS[pr][:, :],
                            func=AF.Ln,
                        ))
                        self_wait()
                        inc(nc.scalar.activation(
                            out=_fap(bvt, pr * 128, [[1, 128]]),
                            in_=_fap(LC, pr * 128, [[1, 128]]),
                            func=AF.Exp,
                            scale=-1.0,
                        ))

            def rowscale_ap(base_t, blocks, off_blk):
                # X[(blk,i,j1,j0)] *= a8[(blk,i,j0)]
                nblk = blocks
                return (
                    _fap(X, off_blk * BLK, [[BLK, nblk], [N, N], [8, 8], [1, 8]]),
                    _fap(base_t, off_blk * N * 8,
                         [[N * 8, nblk], [8, N], [0, 8], [1, 8]]),
                )

            def colscale_ap(base_t, blocks, off_blk):
                nblk = blocks
                return (
                    _fap(X, off_blk * BLK, [[BLK, nblk], [N, N], [1, N]]),
                    _fap(base_t, off_blk * N, [[N, nblk], [0, N], [1, N]]),
                )

            @block.vector
            def _(vector):
                dc = [0]

                def self_wait():
                    if dc[0]:
                        vector.wait_ge(sem_dve, dc[0])

                def inc(inst):
                    inst.then_inc(sem_dve, 1)
                    dc[0] += 1

                vector.wait_ge(sem_in_c, 32)  # G loaded
                for pr in range(npair):
                    vector.wait_ge(sem_act, act_exp_in[2 * pr + 1])
                    self_wait()
                    inc(nc.vector.tensor_mul(
                        _fap(X, pr * 2 * BLK, [[BLK, 2], [N, N], [1, N]]),
                        _fap(X, pr * 2 * BLK, [[BLK, 2], [N, N], [1, N]]),
                        _fap(G, 0, [[0, 2], [N, N], [1, N]]),
                    ))
                for t in range(iters):
                    last = t == iters - 1
                    a8t, bvt = (A8f, Bvf) if last else (A8, Bv)
                    if t == 0:
                        for pr in range(npair):
                            vector.wait_ge(sem_pe, pe_rowsum[(0, pr)])
                            self_wait()
                            inc(nc.vector.reciprocal(
                                _fap(LR, pr * 128, [[1, 128]]),
                                RS[pr][:, :],
                            ))
                    # rowscale units: pairs 0..npair-2 then single block nb-2
                    for u in range(npair):
                        vector.wait_ge(sem_act, act_exp_a8[(t, u)])
                        self_wait()
                        if u < npair - 1:
                            xa, aa = rowscale_ap(a8t, 2, 2 * u)
                        else:
                            xa, aa = rowscale_ap(a8t, 1, nb - 2)
                        inc(nc.vector.tensor_mul(xa, xa, aa))
                    if not last:
                        for u in range(npair):
                            vector.wait_ge(sem_act, act_exp_b[(t, u)])
                            self_wait()
                            if u < npair - 1:
                                xa, ba = colscale_ap(bvt, 2, 2 * u)
                            else:
                                xa, ba = colscale_ap(bvt, 1, nb - 2)
                            inc(nc.vector.tensor_mul(xa, xa, ba))
                    else:
                        for b in range(nb):
                            vector.wait_ge(sem_act, act_exp_b[(t, b // 2)])
                            if b == POOL_BLK:
                                vector.wait_ge(sem_pool, pool_rowscale7[t])
                            if b >= 2:
                                vector.wait_ge(sem_out_p[b % 2], 16 * (b // 2))
                            self_wait()
                            inc(nc.vector.tensor_mul(
                                _fap(stage[b % 2], 0, [[N, N], [1, N]]),
                                _fap(X, b * BLK, [[N, N], [1, N]]),
                                _fap(bvt, b * N, [[0, N], [1, N]]),
                            ))

            @block.gpsimd
            def _(gp):
                qc = [0]

                def self_wait():
                    if qc[0]:
                        gp.wait_ge(sem_pool, qc[0])

                def inc(inst):
                    inst.then_inc(sem_pool, 1)
                    qc[0] += 1

                for t in range(iters):
                    last = t == iters - 1
                    a8t, bvt = (A8f, Bvf) if last else (A8, Bv)
                    gp.wait_ge(sem_act, act_exp_a8[(t, npair - 1)])
                    self_wait()
                    xa, aa = rowscale_ap(a8t, 1, POOL_BLK)
                    inc(nc.gpsimd.tensor_mul(xa, xa, aa))
                    if not last:
                        gp.wait_ge(sem_act, act_exp_b[(t, npair - 1)])
                        self_wait()
                        xa, ba = colscale_ap(bvt, 1, POOL_BLK)
                        inc(nc.gpsimd.tensor_mul(xa, xa, ba))

            @block.tensor
            def _(tensor):
                tensor.wait_ge(sem_in_c, 32)  # ident + gconst loaded

                def chain(psum, src_off, rowsum):
                    out = _fap(psum, 0, [[N, 2], [1, N]])
                    for k in range(N):
                        if rowsum:
                            rhs = _fap(X, src_off + k, [[BLK, 2], [N, N]])
                        else:
                            rhs = _fap(X, src_off + k * N, [[BLK, 2], [1, N]])
                        mm = nc.tensor.matmul(
                            out, identsb[:, :], rhs,
                            start=(k == 0), stop=(k == N - 1),
                        )
                    mm.then_inc(sem_pe, 1)

                for t in range(iters):
                    for pr in range(npair):
                        if t == 0:
                            tensor.wait_ge(sem_dve, dve_gmul[pr])
                        else:
                            tensor.wait_ge(sem_dve, dve_colscale[(t - 1, pr)])
                            if pr == npair - 1:
                                tensor.wait_ge(sem_pool, pool_colscale7[t - 1])
                        chain(RS[pr], pr * 2 * BLK, True)
                    for pr in range(npair):
                        tensor.wait_ge(sem_dve, dve_rowscale[(t, pr)])
                        if pr == npair - 1:
                            tensor.wait_ge(sem_pool, pool_rowscale7[t])
                        chain(CS[pr], pr * 2 * BLK, False)

    return nc


def host_constants(gamma):
    import ml_dtypes

    sg = 1.0 / (1.0 + np.exp(-gamma.astype(np.float64)))
    g = np.exp(sg / TEMP).astype(np.float32).reshape(-1)  # [4096]
    gconst = np.tile(g[None, :], (128, 1)).astype(ml_dtypes.bfloat16)
    ident = np.eye(128, dtype=ml_dtypes.bfloat16)
    return gconst, ident


def make_in_maps(gamma, noise):
    s_per_core = noise.shape[0] // NCORES
    gconst, ident = host_constants(gamma)
    in_maps = []
    for c in range(NCORES):
        shard = np.ascontiguousarray(noise[c * s_per_core : (c + 1) * s_per_core])
        in_maps.append({"noise": shard, "gconst": gconst, "ident": ident})
    return in_maps


def assemble_output(results):
    out = np.concatenate([r["out"] for r in results], axis=0)
    return out.astype(np.float32)


def kernel(gamma: np.ndarray, gumbel_noise: np.ndarray) -> np.ndarray:
    from concourse.bass_utils import run_bass_kernel_spmd

    gamma = np.asarray(gamma, dtype=np.float32)
    noise = np.asarray(gumbel_noise, dtype=np.float32)
    s_per_core = noise.shape[0] // NCORES
    if s_per_core not in _PROGRAM_CACHE:
        _PROGRAM_CACHE[s_per_core] = build_program(s_per_core=s_per_core)
    nc = _PROGRAM_CACHE[s_per_core]

    res = run_bass_kernel_spmd(nc, make_in_maps(gamma, noise), list(range(NCORES)))
    return assemble_output(res.results)


# revision 29
# speedup vs baseline: 11710.9441x; 1.0716x over previous
"""Gumbel-Sinkhorn kernel for Trainium2 (raw Bass, manual sems) — v5.

Math (per sample): L = (sigmoid(gamma)+noise)/temp; 20x row/col normalize in
log space; exp at the end. In linear space with a GLOBAL shift S=80 (any
constant shift cancels in the first row normalization):
    X0 = exp(10*noise - 80) * G,  G = exp(10*sigmoid(gamma))
    repeat 20x:  X /= rowsum(X);  X /= colsum(X)

Layout (sample-per-partition): each core gets 1024 samples as 8 blocks of
128; partition p = sample-in-block, free = (i, j), j innermost (natural DMA
order). X is bf16 throughout (a full row of X0 underflowing bf16 would need
the row-max Gumbel below -1.2: P ~ e^-212; entries far below their row max
flush to 0 harmlessly). Sums accumulate fp32 in PSUM; reciprocals are
compact [p, 64] per block.

Iteration-0 rowsums reach ~2e31, beyond the Scalar-engine Ln range (2^64),
so t=0 uses the exact DVE `reciprocal` (fp32) + an ACT Copy that both
expands x8 and converts to bf16. After the first row normalization all sums
lie in (0, 64] and the ACT Ln/Exp reciprocal path is safe.

Per iteration (pairs of blocks; 4 pairs):
  PE   rowsums: 64 identity-matmuls accumulating X[:, :, j] -> RS [p, 128]
  ACT  ln(RS) -> LR;  exp(-LR) expanded x8 -> A8 bf16  (compact recips)
  DVE  rowscale: X *= A8   (bf16 packed both operands -> 2x mode)
  PE   colsums: 64 identity-matmuls accumulating X[:, i, :] -> CS
  ACT  ln(CS) -> LC;  exp(-LC) -> B bf16 (broadcast over i, innermost packed)
  DVE  colscale: X *= B    (2x mode)

Engine balance: DVE runs a block in 2.13us (bf16 2x), Pool (gpsimd) in
~8.5us. Pool owns block 7 every pass, plus block 6 on odd iterations and in
the final fp32 passes; DVE covers the rest. That balances DVE ~28us/iter
against PE ~28us/iter (pair-granular software pipeline across the engines).
Final iteration uses fp32 scale factors and writes fp32 straight to the
output staging buffer (bf16 everywhere = 1.4e-2 rel err; fp32 final A and B
-> 1.18e-2 on device vs the 2e-2 gate).

Raw Bass with counting semaphores; every compute op increments its engine's
sem by 1; consumers wait the producer's (engine, tick) from a producer map.
DMA completions can reorder across transfers, so staging uses per-parity
semaphores (at most one outstanding transfer per parity) and a dedicated
sem for constants.
"""

import sys

if "/opt/trn_rl_repo" not in sys.path:
    sys.path.insert(0, "/opt/trn_rl_repo")

import numpy as np

N = 64
ITERS = 20
TEMP = 0.1
NUM_SAMPLES = 8192
NCORES = 8
S_PER_CORE = NUM_SAMPLES // NCORES  # 1024
SHIFT = 80.0

_PROGRAM_CACHE = {}


def _fap(t, off, dims):
    """AP on tensor t with partition dim copied from t[:, :] and custom free
    dims (list of [stride, count] in elements)."""
    import concourse.bass as bass

    base = t if isinstance(t, bass.AP) else t[:, :]
    return bass.AP(tensor=base.tensor, offset=base.offset + off, ap=[base.ap[0]] + dims)


def build_program(s_per_core=S_PER_CORE, iters=ITERS):
    from contextlib import ExitStack

    import concourse.bass as bass
    from concourse import mybir

    f32 = mybir.dt.float32
    bf16 = mybir.dt.bfloat16
    AF = mybir.ActivationFunctionType

    nb = s_per_core // 128  # 8 blocks
    npair = nb // 2  # 4 pairs
    BLK = N * N  # 4096
    last_t = iters - 1

    def pool_blocks(t, kind="row"):
        """Blocks the Pool engine scales in pass (t, kind).

        Per pass, DVE runs a block in 2.13us vs Pool's 8.5us; with the PE
        chain barrier per pass the optimum is a constant 7/1 split. The
        final iteration's fp32 rowscale runs DVE at 1x (optimum 6/2), while
        the final colscale feeds the output DMAs, so its tail stays on the
        faster DVE."""
        if nb < 4:
            return [nb - 1]
        if t == last_t:
            return [nb - 2, nb - 1] if kind == "row" else []
        return [nb - 1]

    nc = bass.Bass()
    # register -SHIFT as a const AP so activation(bias=-SHIFT) resolves
    _shift_t = nc.alloc_sbuf_tensor("const-shift", [128, 1], f32)
    nc.gpsimd.memset(_shift_t.ap(), -SHIFT)
    nc.const_aps.aps[(f32, -SHIFT)] = _shift_t.ap()
    nc.all_engine_barrier()

    noise_d = nc.dram_tensor("noise", [s_per_core, N, N], f32, kind="ExternalInput")
    gconst_d = nc.dram_tensor("gconst", [128, BLK], bf16, kind="ExternalInput")
    ident_d = nc.dram_tensor("ident", [128, 128], bf16, kind="ExternalInput")
    out_d = nc.dram_tensor("out", [s_per_core, N, N], f32, kind="ExternalOutput")

    def dram_ap(td, b):
        return bass.AP(
            tensor=td.tensor if hasattr(td, "tensor") else td,
            offset=b * 128 * BLK,
            ap=[[BLK, 128], [1, BLK]],
        )

    with ExitStack() as ctx:
        e = ctx.enter_context
        X = e(nc.sbuf_tensor("x", [128, nb * BLK], bf16))
        stage = [e(nc.sbuf_tensor(f"stage{k}", [128, BLK], f32)) for k in range(2)]
        G = e(nc.sbuf_tensor("g", [128, BLK], bf16))
        identsb = e(nc.sbuf_tensor("identsb", [128, 128], bf16))
        A8 = e(nc.sbuf_tensor("a8", [128, nb * N * 8], bf16))  # [blk, i, 8]
        Bv = e(nc.sbuf_tensor("bv", [128, nb * N], bf16))  # [blk, j]
        A8f = e(nc.sbuf_tensor("a8f", [128, nb * N * 8], f32))
        Bvf = e(nc.sbuf_tensor("bvf", [128, nb * N], f32))
        LR = e(nc.sbuf_tensor("lr", [128, nb * N], f32))  # ln(RS) / recip0
        LC = e(nc.sbuf_tensor("lc", [128, nb * N], f32))
        RS = [e(nc.psum_tensor(f"rs{pr}", [128, 128], f32)) for pr in range(npair)]
        CS = [e(nc.psum_tensor(f"cs{pr}", [128, 128], f32)) for pr in range(npair)]

        sem_in_c = e(nc.semaphore("sem_in_c"))
        sem_in_p = [e(nc.semaphore(f"sem_in_p{k}")) for k in range(2)]
        sem_out_p = [e(nc.semaphore(f"sem_out_p{k}")) for k in range(2)]
        sem_dve = e(nc.semaphore("sem_dve"))
        sem_act = e(nc.semaphore("sem_act"))
        sem_pe = e(nc.semaphore("sem_pe"))
        sem_pool = e(nc.semaphore("sem_pool"))

        # ------- schedule prepass: assign ticks & the scale-producer map ----
        act_exp_in = {b: b + 1 for b in range(nb)}
        _a = nb
        act_exp_a8, act_exp_b = {}, {}
        for _t in range(iters):
            for _pr in range(npair):
                _a += 1 if _t == 0 else 2  # t=0: Copy only; else ln_r+exp_a8
                act_exp_a8[(_t, _pr)] = _a
            for _pr in range(npair):
                _a += 2
                act_exp_b[(_t, _pr)] = _a

        # DVE emits pair units for blocks not owned by Pool; Pool emits
        # single-block units.
        def dve_units(t, kind="row"):
            units = [("pair", pr) for pr in range(npair - 1)]
            if nb - 2 not in pool_blocks(t, kind):
                units.append(("single", nb - 2))
            return units

        dve_gmul = {pr: pr + 1 for pr in range(npair)}
        _d = npair
        dve_recip0 = {}
        dve_scale = {}  # (t, kind, unit) -> tick ; kind in "row"/"col"
        dve_cs19 = {}
        for _t in range(iters):
            if _t == 0:
                for _pr in range(npair):
                    _d += 1
                    dve_recip0[_pr] = _d
            if _t < last_t:
                for _u in dve_units(_t, "row"):
                    _d += 1
                    dve_scale[(_t, "row", _u)] = _d
                for _u in dve_units(_t, "col"):
                    _d += 1
                    dve_scale[(_t, "col", _u)] = _d
            else:
                # lag-1 interleave: rowscale19(pair k) then cs19 of pair
                # k-1's blocks, so the serial output-DMA chain starts early
                # without head-of-line blocking on the first exp_b
                _rus = dve_units(_t, "row")
                for _k, _u in enumerate(_rus):
                    _d += 1
                    dve_scale[(_t, "row", _u)] = _d
                    if _k >= 1 and _rus[_k - 1][0] == "pair":
                        _ppr = _rus[_k - 1][1]
                        for _b in (2 * _ppr, 2 * _ppr + 1):
                            if _b not in pool_blocks(_t, "col"):
                                _d += 1
                                dve_cs19[_b] = _d
                for _b in range(nb):
                    if _b in pool_blocks(_t, "col") or _b in dve_cs19:
                        continue
                    _d += 1
                    dve_cs19[_b] = _d

        pool_scale = {}  # (t, kind, b) -> tick
        pool_cs19 = {}
        _q = 0
        for _t in range(iters):
            for _b in pool_blocks(_t, "row"):
                _q += 1
                pool_scale[(_t, "row", _b)] = _q
            if _t < last_t:
                for _b in pool_blocks(_t, "col"):
                    _q += 1
                    pool_scale[(_t, "col", _b)] = _q
            else:
                for _b in pool_blocks(_t, "col"):
                    _q += 1
                    pool_cs19[_b] = _q

        def producer(t, kind, b):
            """("dve"|"pool", tick) completing scale op `kind` on block b."""
            if b in pool_blocks(t, kind):
                return ("pool", pool_scale[(t, kind, b)])
            if ("single", b) in dve_units(t, kind):
                return ("dve", dve_scale[(t, kind, ("single", b))])
            return ("dve", dve_scale[(t, kind, ("pair", b // 2))])

        pe_rowsum, pe_colsum = {}, {}
        _p = 0
        for _t in range(iters):
            for _pr in range(npair):
                _p += 1
                pe_rowsum[(_t, _pr)] = _p
            for _pr in range(npair):
                _p += 1
                pe_colsum[(_t, _pr)] = _p

        with nc.Block() as block:

            @block.sync
            def _(sync):
                sync.dma_start(out=G[:, :], in_=gconst_d[:, :]).then_inc(sem_in_c, 16)
                sync.dma_start(out=identsb[:, :], in_=ident_d[:, :]).then_inc(
                    sem_in_c, 16
                )
                for b in range(nb):
                    if b >= 2:
                        sync.wait_ge(sem_act, act_exp_in[b - 2])
                        sync.wait_ge(sem_in_p[b % 2], 16 * (b // 2))
                    sync.dma_start(
                        out=stage[b % 2][:, :], in_=dram_ap(noise_d, b)
                    ).then_inc(sem_in_p[b % 2], 16)
                for b in range(nb):
                    if b in pool_blocks(last_t, "col"):
                        sync.wait_ge(sem_pool, pool_cs19[b])
                    else:
                        sync.wait_ge(sem_dve, dve_cs19[b])
                    if b >= 2:
                        sync.wait_ge(sem_out_p[b % 2], 16 * (b // 2))
                    sync.dma_start(
                        out=dram_ap(out_d, b), in_=stage[b % 2][:, :]
                    ).then_inc(sem_out_p[b % 2], 16)
                sync.wait_ge(sem_out_p[0], 16 * (nb - nb // 2))
                sync.wait_ge(sem_out_p[1], 16 * (nb // 2))

            @block.scalar
            def _(scalar):
                ac = [0]

                def self_wait():
                    if ac[0]:
                        scalar.wait_ge(sem_act, ac[0])

                def inc(inst):
                    inst.then_inc(sem_act, 1)
                    ac[0] += 1

                for b in range(nb):
                    scalar.wait_ge(sem_in_p[b % 2], 16 * (b // 2 + 1))
                    self_wait()
                    inc(nc.scalar.activation(
                        out=_fap(X, b * BLK, [[1, BLK]]),
                        in_=stage[b % 2][:, :],
                        func=AF.Exp,
                        scale=10.0,
                        bias=-SHIFT,
                    ))
                for t in range(iters):
                    last = t == last_t
                    a8t = A8
                    bvt = Bvf if last else Bv
                    for pr in range(npair):
                        if t == 0:
                            scalar.wait_ge(sem_dve, dve_recip0[pr])
                            self_wait()
                            inc(nc.scalar.activation(
                                out=_fap(A8, pr * 2 * N * 8,
                                         [[N * 8, 2], [8, N], [1, 8]]),
                                in_=_fap(LR, pr * 128, [[N, 2], [1, N], [0, 8]]),
                                func=AF.Copy,
                            ))
                        else:
                            scalar.wait_ge(sem_pe, pe_rowsum[(t, pr)])
                            self_wait()
                            inc(nc.scalar.activation(
                                out=_fap(LR, pr * 128, [[1, 128]]),
                                in_=RS[pr][:, :],
                                func=AF.Ln,
                            ))
                            self_wait()
                            inc(nc.scalar.activation(
                                out=_fap(a8t, pr * 2 * N * 8,
                                         [[N * 8, 2], [8, N], [1, 8]]),
                                in_=_fap(LR, pr * 128, [[N, 2], [1, N], [0, 8]]),
                                func=AF.Exp,
                                scale=-1.0,
                            ))
                    for pr in range(npair):
                        scalar.wait_ge(sem_pe, pe_colsum[(t, pr)])
                        self_wait()
                        inc(nc.scalar.activation(
                            out=_fap(LC, pr * 128, [[1, 128]]),
                            in_=CS[pr][:, :],
                            func=AF.Ln,
                        ))
                        self_wait()
                        inc(nc.scalar.activation(
                            out=_fap(bvt, pr * 128, [[1, 128]]),
                            in_=_fap(LC, pr * 128, [[1, 128]]),
                            func=AF.Exp,
                            scale=-1.0,
                        ))

            def rowscale_ap(base_t, nblk, off_blk):
                return (
                    _fap(X, off_blk * BLK, [[BLK, nblk], [N, N], [8, 8], [1, 8]]),
                    _fap(base_t, off_blk * N * 8,
                         [[N * 8, nblk], [8, N], [0, 8], [1, 8]]),
                )

            def colscale_ap(base_t, nblk, off_blk):
                return (
                    _fap(X, off_blk * BLK, [[BLK, nblk], [N, N], [1, N]]),
                    _fap(base_t, off_blk * N, [[N, nblk], [0, N], [1, N]]),
                )

            @block.vector
            def _(vector):
                dc = [0]

                def self_wait():
                    if dc[0]:
                        vector.wait_ge(sem_dve, dc[0])

                def inc(inst):
                    inst.then_inc(sem_dve, 1)
                    dc[0] += 1

                vector.wait_ge(sem_in_c, 32)  # G loaded
                for pr in range(npair):
                    vector.wait_ge(sem_act, act_exp_in[2 * pr + 1])
                    self_wait()
                    inc(nc.vector.tensor_mul(
                        _fap(X, pr * 2 * BLK, [[BLK, 2], [N, N], [1, N]]),
                        _fap(X, pr * 2 * BLK, [[BLK, 2], [N, N], [1, N]]),
                        _fap(G, 0, [[0, 2], [N, N], [1, N]]),
                    ))
                for t in range(iters):
                    last = t == last_t
                    a8t = A8
                    bvt = Bvf if last else Bv
                    if t == 0:
                        for pr in range(npair):
                            vector.wait_ge(sem_pe, pe_rowsum[(0, pr)])
                            self_wait()
                            inc(nc.vector.reciprocal(
                                _fap(LR, pr * 128, [[1, 128]]),
                                RS[pr][:, :],
                            ))
                    for u in dve_units(t, "row"):
                        kind, idx = u
                        pr_act = idx // 2 if kind == "single" else idx
                        vector.wait_ge(sem_act, act_exp_a8[(t, pr_act)])
                        self_wait()
                        if kind == "pair":
                            xa, aa = rowscale_ap(a8t, 2, 2 * idx)
                        else:
                            xa, aa = rowscale_ap(a8t, 1, idx)
                        inc(nc.vector.tensor_mul(xa, xa, aa))
                    if not last:
                        for u in dve_units(t, "col"):
                            kind, idx = u
                            pr_act = idx // 2 if kind == "single" else idx
                            vector.wait_ge(sem_act, act_exp_b[(t, pr_act)])
                            self_wait()
                            if kind == "pair":
                                xa, ba = colscale_ap(bvt, 2, 2 * idx)
                            else:
                                xa, ba = colscale_ap(bvt, 1, idx)
                            inc(nc.vector.tensor_mul(xa, xa, ba))
                    else:
                        for b in range(nb):
                            if b in pool_blocks(t, "col"):
                                continue
                            vector.wait_ge(sem_act, act_exp_b[(t, b // 2)])
                            if b >= 2:
                                vector.wait_ge(sem_out_p[b % 2], 16 * (b // 2))
                            self_wait()
                            inc(nc.vector.tensor_mul(
                                _fap(stage[b % 2], 0, [[N, N], [1, N]]),
                                _fap(X, b * BLK, [[N, N], [1, N]]),
                                _fap(bvt, b * N, [[0, N], [1, N]]),
                            ))

            @block.gpsimd
            def _(gp):
                qc = [0]

                def self_wait():
                    if qc[0]:
                        gp.wait_ge(sem_pool, qc[0])

                def inc(inst):
                    inst.then_inc(sem_pool, 1)
                    qc[0] += 1

                for t in range(iters):
                    last = t == last_t
                    a8t = A8
                    bvt = Bvf if last else Bv
                    for b in pool_blocks(t, "row"):
                        gp.wait_ge(sem_act, act_exp_a8[(t, b // 2)])
                        self_wait()
                        xa, aa = rowscale_ap(a8t, 1, b)
                        inc(nc.gpsimd.tensor_mul(xa, xa, aa))
                    if not last:
                        for b in pool_blocks(t, "col"):
                            gp.wait_ge(sem_act, act_exp_b[(t, b // 2)])
                            self_wait()
                            xa, ba = colscale_ap(bvt, 1, b)
                            inc(nc.gpsimd.tensor_mul(xa, xa, ba))
                    else:
                        for b in pool_blocks(t, "col"):
                            gp.wait_ge(sem_act, act_exp_b[(t, b // 2)])
                            if b >= 2:
                                gp.wait_ge(sem_out_p[b % 2], 16 * (b // 2))
                            self_wait()
                            inc(nc.gpsimd.tensor_mul(
                                _fap(stage[b % 2], 0, [[N, N], [1, N]]),
                                _fap(X, b * BLK, [[N, N], [1, N]]),
                                _fap(bvt, b * N, [[0, N], [1, N]]),
                            ))

            @block.tensor
            def _(tensor):
                tensor.wait_ge(sem_in_c, 32)  # ident + gconst loaded

                def chain(psum, src_off, rowsum):
                    out = _fap(psum, 0, [[N, 2], [1, N]])
                    for k in range(N):
                        if rowsum:
                            rhs = _fap(X, src_off + k, [[BLK, 2], [N, N]])
                        else:
                            rhs = _fap(X, src_off + k * N, [[BLK, 2], [1, N]])
                        mm = nc.tensor.matmul(
                            out, identsb[:, :], rhs,
                            start=(k == 0), stop=(k == N - 1),
                        )
                    mm.then_inc(sem_pe, 1)

                def wait_scale(t, kind, blocks):
                    # one wait per engine: engines are in-order, so waiting
                    # the max tick per engine covers all listed blocks
                    per_engine = {}
                    for b in blocks:
                        sem_name, tick = producer(t, kind, b)
                        per_engine[sem_name] = max(per_engine.get(sem_name, 0), tick)
                    for sem_name, tick in per_engine.items():
                        tensor.wait_ge(
                            sem_pool if sem_name == "pool" else sem_dve, tick
                        )

                for t in range(iters):
                    for pr in range(npair):
                        blocks = [2 * pr, 2 * pr + 1]
                        if t == 0:
                            tensor.wait_ge(sem_dve, dve_gmul[pr])
                        else:
                            wait_scale(t - 1, "col", blocks)
                        chain(RS[pr], pr * 2 * BLK, True)
                    for pr in range(npair):
                        blocks = [2 * pr, 2 * pr + 1]
                        wait_scale(t, "row", blocks)
                        chain(CS[pr], pr * 2 * BLK, False)

    return nc


def host_constants(gamma):
    import ml_dtypes

    sg = 1.0 / (1.0 + np.exp(-gamma.astype(np.float64)))
    g = np.exp(sg / TEMP).astype(np.float32).reshape(-1)  # [4096]
    gconst = np.tile(g[None, :], (128, 1)).astype(ml_dtypes.bfloat16)
    ident = np.eye(128, dtype=ml_dtypes.bfloat16)
    return gconst, ident


def make_in_maps(gamma, noise):
    s_per_core = noise.shape[0] // NCORES
    gconst, ident = host_constants(gamma)
    in_maps = []
    for c in range(NCORES):
        shard = np.ascontiguousarray(noise[c * s_per_core : (c + 1) * s_per_core])
        in_maps.append({"noise": shard, "gconst": gconst, "ident": ident})
    return in_maps


def assemble_output(results):
    out = np.concatenate([r["out"] for r in results], axis=0)
    return out.astype(np.float32)


def kernel(gamma: np.ndarray, gumbel_noise: np.ndarray) -> np.ndarray:
    from concourse.bass_utils import run_bass_kernel_spmd

    gamma = np.asarray(gamma, dtype=np.float32)
    noise = np.asarray(gumbel_noise, dtype=np.float32)
    s_per_core = noise.shape[0] // NCORES
    if s_per_core not in _PROGRAM_CACHE:
        _PROGRAM_CACHE[s_per_core] = build_program(s_per_core=s_per_core)
    nc = _PROGRAM_CACHE[s_per_core]

    res = run_bass_kernel_spmd(nc, make_in_maps(gamma, noise), list(range(NCORES)))
    return assemble_output(res.results)
